# revision 7
# baseline (speedup 1.0000x reference)
"""CapsNet dynamic-routing kernel for Trainium2 (8 NeuronCores, batch-parallel).

Restructured routing that never materializes u_hat (B=256,D=10,M=32,P=36,I=8,O=16):
  y[b,d,m,i] = sum_p c[b,d,m,p] x[b,m,p,i]        (op A, PE per (b,chunk))
  s[b,d,o]   = sum_{m,i} y[b,d,m,i] W[d,m,o,i]    (op B)
  g[b,d,m,i] = sum_o W[d,m,o,i] v[b,d,o]          (op C)
  b[b,d,m,p]+= sum_i x[b,p,i] g[b,d,m,i]          (op D, PE per (b,chunk))

m is split into 11 chunks of 3, grouped in 4 triples t (chunks 3t..3t+L-1).
b-state/c live on SBUF rows (mc,p)=108, one tile per triple with cols
(pos,b,d). y/g/z rows are (mc,i)=24 per chunk at row offsets 0/32/64 with
zero padding rows. All x-derived matmul operands are prepared host-side and
shipped in a handful of packed DMAs (small operands first so op B of the
uniform-c iteration starts immediately).

PE operands are bf16 (fp32 matmuls cost 2x LDWEIGHTS+MATMUL passes); PSUM
accumulation, b-state and squash stay fp32. The scalar engine only uses
Copy/Square/Ln/Exp; sqrt comes from exp(-.5 ln). A dummy ln issued right
after each softmax's exps pulls the ln-side activation-table load off the
squash critical path. Softmaxes are processed per-triple so the PE pipeline
overlaps the vector work; PSUM->SBUF casts run on scalar, softmax divides
use the fast DVE reciprocal approximation, and the normalization multiplies
alternate vector/gpsimd.
"""

import numpy as np
import ml_dtypes

B, D, M, P, I, O = 256, 10, 32, 36, 8, 16
NCORES = 8
BC = B // NCORES
NCH = 11
NT = 4                      # triples of chunks: [0-2],[3-5],[6-8],[9-10]
TLEN = [3, 3, 3, 2]
TROWS = [96, 96, 96, 64]    # 32 rows per chunk (24 real + 8 zero pad)
EPS = 1e-7

BF16 = ml_dtypes.bfloat16

# packed free-dim offsets (in elements) for the merged DMA tensors
SW_XS = [t * 32 for t in range(NT)]                     # [96, 128]
SW_WS = [128 + t * D * O for t in range(NT)]            # [96, 128+640]
SW_COLS = 128 + NT * D * O
WC_OFF = [0, 960, 1920, 2880]                           # [16, 3520]
WC_COLS = 3520
X2_OFF = [t * BC * 3 * P for t in range(NT)]            # [96, 4*3456]
X2_COLS = NT * BC * 3 * P


def _host_prep(xc, Wd):
    """Per-core host-side tensor prep. xc: [BC,M,P,I], Wd: [D,M,O,I]."""
    f32 = np.float32
    xbd = np.zeros((3, P, BC, NCH, 3, I), f32)
    xsum = xc.sum(axis=2) * (1.0 / D)
    small = np.zeros((96, SW_COLS), f32)
    wcp = np.zeros((O, WC_COLS), f32)
    x2p = np.zeros((96, X2_COLS), f32)
    for t in range(NT):
        L = TLEN[t]
        rows = TROWS[t]
        t2 = np.zeros((rows, BC, 3, P), f32)
        tw = np.zeros((rows, D, O), f32)
        tcc = np.zeros((O, D, L, 4, I), f32)
        txs = np.zeros((rows, BC), f32)
        for pos in range(L):
            c = 3 * t + pos
            r0 = 32 * pos
            for mc in range(3):
                m = 3 * c + mc
                if m >= M:
                    continue
                xmi = xc[:, m, :, :]                      # [b, p, i]
                xbd[mc, :, :, c, mc, :] = xmi.transpose(1, 0, 2)
                rr = r0 + 8 * mc
                t2[rr:rr + 8, :, mc, :] = xmi.transpose(2, 0, 1)
                tw[rr:rr + 8, :, :] = Wd[:, m, :, :].transpose(2, 0, 1)
                tcc[:, :, pos, mc, :] = Wd[:, m, :, :].transpose(1, 0, 2)
                txs[rr:rr + 8, :] = xsum[:, m, :].T
        small[:rows, SW_XS[t]:SW_XS[t] + BC] = txs
        small[:rows, SW_WS[t]:SW_WS[t] + D * O] = tw.reshape(rows, D * O)
        wcp[:, WC_OFF[t]:WC_OFF[t] + D * L * 32] = tcc.reshape(O, D * L * 32)
        x2p[:rows, X2_OFF[t]:X2_OFF[t] + BC * 3 * P] = t2.reshape(
            rows, BC * 3 * P)
    out = {
        "smallp": small.astype(BF16),
        "wcp": wcp.astype(BF16),
        "x2p": np.ascontiguousarray(x2p.astype(BF16)),
        "xbd": np.ascontiguousarray(
            xbd.reshape(108, BC * NCH * 24).astype(BF16)),
        "ident": np.eye(32, dtype=f32),
    }
    ones_bd = np.zeros((3, P, 3, I), f32)
    for mc in range(3):
        ones_bd[mc, :, mc, :] = 1.0
    out["ones_bd"] = ones_bd.reshape(108, 24).astype(BF16)
    return out


def _build(nc):
    import concourse.mybir as mybir
    import concourse.tile as tile

    f32 = mybir.dt.float32
    bf16 = mybir.dt.bfloat16
    AF = mybir.ActivationFunctionType
    AX = mybir.AxisListType
    ALU = mybir.AluOpType

    ins = {}
    # declaration order == DMA issue order: small it0-critical operands
    # first, then the two big x-derived tensors.
    specs = [
        ("smallp", [96, SW_COLS], bf16),
        ("wcp", [O, WC_COLS], bf16),
        ("ones_bd", [108, 24], bf16),
        ("ident", [32, 32], f32),
        ("x2p", [96, X2_COLS], bf16),
        ("xbd", [108, BC * NCH * 24], bf16),
    ]
    for name, shape, dt in specs:
        ins[name] = nc.declare_dram_parameter(name, shape, dt, isOutput=False)
    out_d = nc.declare_dram_parameter("out_v", [BC, D * O], f32, isOutput=True)

    with tile.TileContext(nc) as tc:
        with (
            tc.tile_pool(name="const", bufs=1) as cpool,
            tc.tile_pool(name="state", bufs=1) as spool,
            tc.tile_pool(name="small", bufs=2) as mpool,
            tc.tile_pool(name="psA", bufs=6, space="PSUM") as psA,
            tc.tile_pool(name="psS", bufs=2, space="PSUM") as psS,
        ):
            sb = {}
            for name, t in ins.items():
                st = cpool.tile(list(t.shape), t.dtype, name=name, tag=name)
                nc.sync.dma_start(st[:], t[:])
                sb[name] = st

            xbd_r = sb["xbd"][:].rearrange("r (b c k) -> r b c k", b=BC, c=NCH)
            xbd2 = [sb["x2p"][0:TROWS[t], X2_OFF[t]:X2_OFF[t] + BC * 3 * P]
                    .rearrange("r (b q) -> r b q", b=BC) for t in range(NT)]
            ws = [sb["smallp"][0:TROWS[t], SW_WS[t]:SW_WS[t] + D * O]
                  .rearrange("r (d o) -> r d o", d=D) for t in range(NT)]
            wc = [sb["wcp"][:, WC_OFF[t]:WC_OFF[t] + D * TLEN[t] * 32]
                  .rearrange("o (d r) -> o d r", d=D) for t in range(NT)]
            xs = [sb["smallp"][0:TROWS[t], SW_XS[t]:SW_XS[t] + BC]
                  for t in range(NT)]

            # per-triple routing state: [108, L*BC*D]
            bstate = [spool.tile([108, TLEN[t] * BC * D], f32,
                                 name=f"bst{t}", tag=f"bst{t}")
                      for t in range(NT)]
            bst = [bstate[t][:].rearrange("r (c b d) -> r c b d",
                                          c=TLEN[t], b=BC)
                   for t in range(NT)]
            ctile = [spool.tile([108, TLEN[t] * BC * D], bf16,
                                name=f"ct{t}", tag=f"ct{t}")
                     for t in range(NT)]
            ct = [ctile[t][:].rearrange("r (c b d) -> r c b d",
                                        c=TLEN[t], b=BC)
                  for t in range(NT)]
            ytiles = [spool.tile([TROWS[t], BC * D], bf16, tag=f"y{t}",
                                 name=f"y{t}") for t in range(NT)]
            gtiles = [spool.tile([TROWS[t], BC * D], bf16, tag=f"g{t}",
                                 name=f"g{t}") for t in range(NT)]
            ztiles = [spool.tile([TROWS[t], BC * D], f32, tag=f"z{t}",
                                 name=f"z{t}") for t in range(NT)]
            dumml = spool.tile([1, 2], f32, name="dumml", tag="dumml")
            for t in range(NT):
                nc.gpsimd.memset(ytiles[t][:], 0.0)   # pad rows must stay 0
                nc.gpsimd.memset(ztiles[t][:], 1.0)   # pad rows must stay 1
            nc.gpsimd.memset(dumml[:], 1.0)

            def dummy_ln():
                # pulls the ln-side ACT_TABLE_LOAD off the squash critical
                # path: issued while the PE grinds, so the later real ln
                # needs no table switch.
                nc.scalar.activation(dumml[:, 0:1], dumml[:, 1:2], AF.Ln)

            def op_B(src_y, it):
                s_ps = psS.tile([BC, D * O], f32, tag="s", name="s_ps")
                for d in range(D):
                    for t in range(NT):
                        if it == 0:
                            lhsT = xs[t]
                        else:
                            lhsT = src_y[t][:].rearrange(
                                "r (b d) -> r d b", d=D)[:, d, :]
                        nc.tensor.matmul(
                            s_ps[:, d * O:(d + 1) * O], lhsT, ws[t][:, d, :],
                            start=(t == 0), stop=(t == NT - 1))
                return s_ps

            def squash(s_ps):
                # v = s * ssum / ((1+ssum) sqrt(ssum+eps)); sqrt via exp/ln
                # (keeps the scalar engine inside one activation-table set).
                s_sb = mpool.tile([BC, D * O], f32, tag="ssb", name="s_sb")
                nc.scalar.copy(s_sb[:], s_ps[:])
                sq = mpool.tile([BC, D * O], f32, tag="sq", name="sq")
                nc.scalar.activation(sq[:], s_ps[:], AF.Square)
                ssum = mpool.tile([BC, D], f32, tag="ssum", name="ssum")
                nc.vector.tensor_reduce(
                    ssum[:], sq[:].rearrange("b (d o) -> b d o", d=D),
                    axis=AX.X, op=ALU.add)
                se = mpool.tile([BC, D], f32, tag="se", name="se")
                nc.vector.tensor_scalar_add(se[:], ssum[:], EPS)
                lt = mpool.tile([BC, D], f32, tag="lt", name="lt")
                nc.scalar.activation(lt[:], se[:], AF.Ln)
                rs = mpool.tile([BC, D], f32, tag="rs", name="rs")
                nc.scalar.activation(rs[:], lt[:], AF.Exp, scale=-0.5)
                den = mpool.tile([BC, D], f32, tag="den", name="den")
                nc.vector.tensor_scalar_add(den[:], ssum[:], 1.0)
                rden = mpool.tile([BC, D], f32, tag="rden", name="rden")
                nc.vector.reciprocal_approx_fast(rden[:], den[:])
                sc = mpool.tile([BC, D], f32, tag="sc", name="sc")
                nc.vector.tensor_mul(sc[:], ssum[:], rden[:])
                nc.vector.tensor_mul(sc[:], sc[:], rs[:])
                v = mpool.tile([BC, D * O], f32, tag="v", name="v")
                nc.vector.tensor_mul(
                    v[:].rearrange("b (d o) -> b d o", d=D),
                    s_sb[:].rearrange("b (d o) -> b d o", d=D),
                    sc[:].broadcast_to([BC, D, O]))
                return v

            def op_CD(v, it):
                # stage VT as [o=16, (d,b)] so matmul reads start at partition 0
                vtp = mpool.tile([O, D * 32], bf16, tag="vtp", name="vtp")
                for d in range(D):
                    vt_ps = psA.tile([O, 32], f32, tag="ps", name="vt_ps")
                    nc.tensor.transpose(
                        vt_ps[:], v[:, 16 * d:16 * d + 16], sb["ident"][:])
                    nc.vector.tensor_copy(vtp[:, 32 * d:32 * d + 32], vt_ps[:])
                for t in range(NT):
                    L = TLEN[t]
                    g_ps = psA.tile([32 * L, D * 32], f32, tag="ps",
                                    name="g_ps")
                    for d in range(D):
                        nc.tensor.matmul(
                            g_ps[:, 32 * d:32 * d + 32], wc[t][:, d, :],
                            vtp[:, 32 * d:32 * d + 32], start=True, stop=True)
                    nc.scalar.copy(
                        gtiles[t][:].rearrange("r (b d) -> r b d", b=BC),
                        g_ps[:].rearrange("r (d b) -> r b d", d=D))
                for c in range(NCH):
                    t, pos = c // 3, c % 3
                    r0 = 32 * pos
                    d_ps = psA.tile([108, BC * D], f32, tag="ps", name="d_ps")
                    for b in range(BC):
                        nc.tensor.matmul(
                            d_ps[:, D * b:D * b + D],
                            xbd2[t][r0:r0 + 24, b, :],
                            gtiles[t][r0:r0 + 24, D * b:D * b + D],
                            start=True, stop=True)
                    dst = bst[t][:, pos, :, :].rearrange("r b d -> r (b d)")
                    if it == 0:
                        nc.vector.tensor_copy(dst, d_ps[:])
                    else:
                        nc.vector.tensor_add(dst, dst, d_ps[:])

            def op_A_group(t, dst_tiles):
                # op A for the chunks of triple t; y casts on scalar so the
                # vector queue stays free for the softmax chains.
                L = TLEN[t]
                for pos in range(L):
                    c = 3 * t + pos
                    y_ps = psA.tile([24, BC * D], f32, tag="ps", name="y_ps")
                    for b in range(BC):
                        nc.tensor.matmul(
                            y_ps[:, D * b:D * b + D],
                            xbd_r[:, b, c, :],
                            ct[t][:, pos, b, :],
                            start=True, stop=True)
                    nc.scalar.copy(
                        dst_tiles[t][32 * pos:32 * pos + 24, :], y_ps[:])

            def softmax_d_group(t):
                # softmax over d on triple t's b-state -> ct[t] (bf16);
                # the exp itself is hoisted by the caller so the scalar
                # queue is not blocked behind PE-dependent casts.
                L = TLEN[t]
                zs = mpool.tile([108, L * BC], f32, tag=f"zs{t}",
                                name=f"zs{t}")
                nc.vector.tensor_reduce(zs[:], ct[t], axis=AX.X, op=ALU.add)
                zr = mpool.tile([108, L * BC], f32, tag=f"zr{t}",
                                name=f"zr{t}")
                nc.vector.reciprocal_approx_fast(zr[:], zs[:])
                eng = nc.vector if t % 2 == 0 else nc.gpsimd
                eng.tensor_mul(
                    ct[t], ct[t],
                    zr[:].rearrange("r (c b) -> r c b", c=L)
                    .broadcast_to([108, L, BC, D]))

            # ---- iteration 1 (c uniform = 1/D, folded into xs) ----
            s_ps = op_B(None, it=0)
            v = squash(s_ps)
            op_CD(v, it=0)

            # ---- iteration 2: softmax over d, pipelined per triple ----
            for t in range(NT):
                nc.scalar.activation(ctile[t][:], bstate[t][:], AF.Exp)
            dummy_ln()
            for t in range(NT):
                softmax_d_group(t)
                op_A_group(t, ytiles)
            s_ps = op_B(ytiles, it=1)
            v = squash(s_ps)
            op_CD(v, it=1)

            # ---- final: softmax over p fused into op A, per triple ----
            for t in range(NT):
                nc.scalar.activation(ctile[t][:], bstate[t][:], AF.Exp)
            dummy_ln()
            for t in range(NT):
                L = TLEN[t]
                for pos in range(L):
                    z_ps = psA.tile([24, BC * D], f32, tag="ps", name="z_ps")
                    nc.tensor.matmul(
                        z_ps[:], sb["ones_bd"][:],
                        ct[t][:, pos, :, :].rearrange("r b d -> r (b d)"),
                        start=True, stop=True)
                    nc.scalar.copy(
                        ztiles[t][32 * pos:32 * pos + 24, :], z_ps[:])
                op_A_group(t, ytiles)
                zrt = spool.tile([TROWS[t], BC * D], f32, tag=f"zr_t{t}",
                                 name=f"zr_t{t}")
                nc.vector.reciprocal_approx_fast(zrt[:], ztiles[t][:])
                nc.gpsimd.tensor_mul(ytiles[t][:], ytiles[t][:], zrt[:])
            s_ps = op_B(ytiles, it=2)
            v = squash(s_ps)
            nc.sync.dma_start(out_d[:], v[:])
    return nc


_CACHE = {}


def kernel(x, W):
    import sys
    if "/opt/trn_rl_repo" not in sys.path:
        sys.path.insert(0, "/opt/trn_rl_repo")
    from concourse import bass_utils

    x = np.asarray(x, np.float32)
    Wd = np.asarray(W, np.float32)[0, :, :, 0]  # [D,M,O,I]
    if "nc" not in _CACHE:
        from concourse import bacc
        nc = _build(bacc.Bacc(None, target_bir_lowering=False))
        nc.compile()
        _CACHE["nc"] = nc
    nc = _CACHE["nc"]
    in_maps = [_host_prep(x[k * BC:(k + 1) * BC], Wd) for k in range(NCORES)]
    res = bass_utils.run_bass_kernel_spmd(nc, in_maps, list(range(NCORES)))
    outs = [res.results[k]["out_v"].reshape(BC, D, O) for k in range(NCORES)]
    return np.concatenate(outs, axis=0)


# revision 8
# speedup vs baseline: 1.0112x; 1.0112x over previous
"""CapsNet dynamic-routing kernel for Trainium2 (8 NeuronCores, batch-parallel).

Restructured routing that never materializes u_hat (B=256,D=10,M=32,P=36,I=8,O=16):
  y[b,d,m,i] = sum_p c[b,d,m,p] x[b,m,p,i]        (op A, PE per (b,chunk))
  s[b,d,o]   = sum_{m,i} y[b,d,m,i] W[d,m,o,i]    (op B)
  g[b,d,m,i] = sum_o W[d,m,o,i] v[b,d,o]          (op C)
  b[b,d,m,p]+= sum_i x[b,p,i] g[b,d,m,i]          (op D, PE per (b,chunk))

m is split into 11 chunks of 3, grouped in 4 triples t (chunks 3t..3t+L-1).
b-state/c live on SBUF rows (mc,p)=108, one tile per triple with cols
(pos,b,d). y/g/z rows are (mc,i)=24 per chunk at row offsets 0/32/64 with
zero padding rows. All x-derived matmul operands are prepared host-side and
shipped in a handful of packed DMAs (small operands first so op B of the
uniform-c iteration starts immediately).

PE operands are bf16 (fp32 matmuls cost 2x LDWEIGHTS+MATMUL passes); PSUM
accumulation, b-state and squash stay fp32. The scalar engine only uses
Copy/Square/Ln/Exp; sqrt comes from exp(-.5 ln). A dummy ln issued right
after each softmax's exps pulls the ln-side activation-table load off the
squash critical path. Softmaxes are processed per-triple so the PE pipeline
overlaps the vector work; PSUM->SBUF casts run on scalar, softmax divides
use the fast DVE reciprocal approximation, and the normalization multiplies
alternate vector/gpsimd.
"""

import numpy as np
import ml_dtypes

B, D, M, P, I, O = 256, 10, 32, 36, 8, 16
NCORES = 8
BC = B // NCORES
NCH = 11
NT = 4                      # triples of chunks: [0-2],[3-5],[6-8],[9-10]
TLEN = [3, 3, 3, 2]
TROWS = [96, 96, 96, 64]    # 32 rows per chunk (24 real + 8 zero pad)
EPS = 1e-7

BF16 = ml_dtypes.bfloat16

# packed free-dim offsets (in elements) for the merged DMA tensors
SW_XS = [t * 32 for t in range(NT)]                     # [96, 128]
SW_WS = [128 + t * D * O for t in range(NT)]            # [96, 128+640]
SW_COLS = 128 + NT * D * O
WC_OFF = [0, 960, 1920, 2880]                           # [16, 3520]
WC_COLS = 3520
X2_OFF = [t * BC * 3 * P for t in range(NT)]            # [96, 4*3456]
X2_COLS = NT * BC * 3 * P


def _host_prep(xc, Wd):
    """Per-core host-side tensor prep. xc: [BC,M,P,I], Wd: [D,M,O,I]."""
    f32 = np.float32
    xbd = np.zeros((3, P, BC, NCH, 3, I), f32)
    xsum = xc.sum(axis=2) * (1.0 / D)
    small = np.zeros((96, SW_COLS), f32)
    wcp = np.zeros((O, WC_COLS), f32)
    x2p = np.zeros((96, X2_COLS), f32)
    for t in range(NT):
        L = TLEN[t]
        rows = TROWS[t]
        t2 = np.zeros((rows, BC, 3, P), f32)
        tw = np.zeros((rows, D, O), f32)
        tcc = np.zeros((O, D, L, 4, I), f32)
        txs = np.zeros((rows, BC), f32)
        for pos in range(L):
            c = 3 * t + pos
            r0 = 32 * pos
            for mc in range(3):
                m = 3 * c + mc
                if m >= M:
                    continue
                xmi = xc[:, m, :, :]                      # [b, p, i]
                xbd[mc, :, :, c, mc, :] = xmi.transpose(1, 0, 2)
                rr = r0 + 8 * mc
                t2[rr:rr + 8, :, mc, :] = xmi.transpose(2, 0, 1)
                tw[rr:rr + 8, :, :] = Wd[:, m, :, :].transpose(2, 0, 1)
                tcc[:, :, pos, mc, :] = Wd[:, m, :, :].transpose(1, 0, 2)
                txs[rr:rr + 8, :] = xsum[:, m, :].T
        small[:rows, SW_XS[t]:SW_XS[t] + BC] = txs
        small[:rows, SW_WS[t]:SW_WS[t] + D * O] = tw.reshape(rows, D * O)
        wcp[:, WC_OFF[t]:WC_OFF[t] + D * L * 32] = tcc.reshape(O, D * L * 32)
        x2p[:rows, X2_OFF[t]:X2_OFF[t] + BC * 3 * P] = t2.reshape(
            rows, BC * 3 * P)
    out = {
        "smallp": small.astype(BF16),
        "wcp": wcp.astype(BF16),
        "x2p": np.ascontiguousarray(x2p.astype(BF16)),
        "xbd": np.ascontiguousarray(
            xbd.reshape(108, BC * NCH * 24).astype(BF16)),
        "ident": np.eye(32, dtype=f32),
    }
    ones_bd = np.zeros((3, P, 3, I), f32)
    for mc in range(3):
        ones_bd[mc, :, mc, :] = 1.0
    out["ones_bd"] = ones_bd.reshape(108, 24).astype(BF16)
    return out


def _build(nc):
    import concourse.mybir as mybir
    import concourse.tile as tile

    f32 = mybir.dt.float32
    i32 = mybir.dt.int32
    bf16 = mybir.dt.bfloat16
    AF = mybir.ActivationFunctionType
    AX = mybir.AxisListType
    ALU = mybir.AluOpType

    ins = {}
    # declaration order == DMA issue order: small it0-critical operands
    # first, then the two big x-derived tensors.
    specs = [
        ("smallp", [96, SW_COLS], bf16),
        ("wcp", [O, WC_COLS], bf16),
        ("ones_bd", [108, 24], bf16),
        ("ident", [32, 32], f32),
        ("x2p", [96, X2_COLS], bf16),
        ("xbd", [108, BC * NCH * 24], bf16),
    ]
    for name, shape, dt in specs:
        ins[name] = nc.declare_dram_parameter(name, shape, dt, isOutput=False)
    out_d = nc.declare_dram_parameter("out_v", [BC, D * O], f32, isOutput=True)

    with tile.TileContext(nc) as tc:
        with (
            tc.tile_pool(name="const", bufs=1) as cpool,
            tc.tile_pool(name="state", bufs=1) as spool,
            tc.tile_pool(name="small", bufs=2) as mpool,
            tc.tile_pool(name="psA", bufs=6, space="PSUM") as psA,
            tc.tile_pool(name="psS", bufs=2, space="PSUM") as psS,
        ):
            sb = {}
            for name, t in ins.items():
                st = cpool.tile(list(t.shape), t.dtype, name=name, tag=name)
                nc.sync.dma_start(st[:], t[:])
                sb[name] = st

            xbd_r = sb["xbd"][:].rearrange("r (b c k) -> r b c k", b=BC, c=NCH)
            xbd2 = [sb["x2p"][0:TROWS[t], X2_OFF[t]:X2_OFF[t] + BC * 3 * P]
                    .rearrange("r (b q) -> r b q", b=BC) for t in range(NT)]
            ws = [sb["smallp"][0:TROWS[t], SW_WS[t]:SW_WS[t] + D * O]
                  .rearrange("r (d o) -> r d o", d=D) for t in range(NT)]
            wc = [sb["wcp"][:, WC_OFF[t]:WC_OFF[t] + D * TLEN[t] * 32]
                  .rearrange("o (d r) -> o d r", d=D) for t in range(NT)]
            xs = [sb["smallp"][0:TROWS[t], SW_XS[t]:SW_XS[t] + BC]
                  for t in range(NT)]

            # per-triple routing state: [108, L*BC*D]
            bstate = [spool.tile([108, TLEN[t] * BC * D], f32,
                                 name=f"bst{t}", tag=f"bst{t}")
                      for t in range(NT)]
            bst = [bstate[t][:].rearrange("r (c b d) -> r c b d",
                                          c=TLEN[t], b=BC)
                   for t in range(NT)]
            ctile = [spool.tile([108, TLEN[t] * BC * D], bf16,
                                name=f"ct{t}", tag=f"ct{t}")
                     for t in range(NT)]
            ct = [ctile[t][:].rearrange("r (c b d) -> r c b d",
                                        c=TLEN[t], b=BC)
                  for t in range(NT)]
            ytiles = [spool.tile([TROWS[t], BC * D], bf16, tag=f"y{t}",
                                 name=f"y{t}") for t in range(NT)]
            gtiles = [spool.tile([TROWS[t], BC * D], bf16, tag=f"g{t}",
                                 name=f"g{t}") for t in range(NT)]
            ztiles = [spool.tile([TROWS[t], BC * D], f32, tag=f"z{t}",
                                 name=f"z{t}") for t in range(NT)]
            for t in range(NT):
                nc.gpsimd.memset(ytiles[t][:], 0.0)   # pad rows must stay 0
                nc.gpsimd.memset(ztiles[t][:], 1.0)   # pad rows must stay 1

            def op_B(src_y, it):
                s_ps = psS.tile([BC, D * O], f32, tag="s", name="s_ps")
                for d in range(D):
                    for t in range(NT):
                        if it == 0:
                            lhsT = xs[t]
                        else:
                            lhsT = src_y[t][:].rearrange(
                                "r (b d) -> r d b", d=D)[:, d, :]
                        nc.tensor.matmul(
                            s_ps[:, d * O:(d + 1) * O], lhsT, ws[t][:, d, :],
                            start=(t == 0), stop=(t == NT - 1))
                return s_ps

            def squash(s_ps):
                # v = s * ssum / ((1+ssum) sqrt(ssum+eps)); sqrt via exp/ln
                # (keeps the scalar engine inside one activation-table set).
                s_sb = mpool.tile([BC, D * O], f32, tag="ssb", name="s_sb")
                nc.scalar.copy(s_sb[:], s_ps[:])
                sq = mpool.tile([BC, D * O], f32, tag="sq", name="sq")
                nc.scalar.activation(sq[:], s_ps[:], AF.Square)
                ssum = mpool.tile([BC, D], f32, tag="ssum", name="ssum")
                nc.vector.tensor_reduce(
                    ssum[:], sq[:].rearrange("b (d o) -> b d o", d=D),
                    axis=AX.X, op=ALU.add)
                se = mpool.tile([BC, D], f32, tag="se", name="se")
                nc.vector.tensor_scalar_add(se[:], ssum[:], EPS)
                # rs = rsqrt(se) via shift/magic seed + 2 Newton steps --
                # all on DVE, so the scalar engine never leaves the exp
                # activation-table set (zero mid-kernel table reloads).
                rs = mpool.tile([BC, D], f32, tag="rs", name="rs")
                nc.vector.tensor_scalar(
                    rs[:].bitcast(i32), se[:].bitcast(i32), 1, None,
                    op0=ALU.logical_shift_right)
                nc.vector.tensor_scalar(
                    rs[:].bitcast(i32), rs[:].bitcast(i32), -1, 0x5F3759DF,
                    op0=ALU.mult, op1=ALU.add)
                nt = mpool.tile([BC, D], f32, tag="nt", name="nt")
                for _ in range(2):
                    nc.vector.tensor_mul(nt[:], rs[:], rs[:])
                    nc.vector.tensor_mul(nt[:], nt[:], se[:])
                    nc.vector.tensor_scalar(nt[:], nt[:], -0.5, 1.5,
                                            op0=ALU.mult, op1=ALU.add)
                    nc.vector.tensor_mul(rs[:], rs[:], nt[:])
                den = mpool.tile([BC, D], f32, tag="den", name="den")
                nc.vector.tensor_scalar_add(den[:], ssum[:], 1.0)
                rden = mpool.tile([BC, D], f32, tag="rden", name="rden")
                nc.vector.reciprocal_approx_fast(rden[:], den[:])
                sc = mpool.tile([BC, D], f32, tag="sc", name="sc")
                nc.vector.tensor_mul(sc[:], ssum[:], rden[:])
                nc.vector.tensor_mul(sc[:], sc[:], rs[:])
                v = mpool.tile([BC, D * O], f32, tag="v", name="v")
                nc.vector.tensor_mul(
                    v[:].rearrange("b (d o) -> b d o", d=D),
                    s_sb[:].rearrange("b (d o) -> b d o", d=D),
                    sc[:].broadcast_to([BC, D, O]))
                return v

            def op_CD(v, it):
                # stage VT as [o=16, (d,b)] so matmul reads start at partition 0
                vtp = mpool.tile([O, D * 32], bf16, tag="vtp", name="vtp")
                for d in range(D):
                    vt_ps = psA.tile([O, 32], f32, tag="ps", name="vt_ps")
                    nc.tensor.transpose(
                        vt_ps[:], v[:, 16 * d:16 * d + 16], sb["ident"][:])
                    nc.vector.tensor_copy(vtp[:, 32 * d:32 * d + 32], vt_ps[:])
                for t in range(NT):
                    L = TLEN[t]
                    g_ps = psA.tile([32 * L, D * 32], f32, tag="ps",
                                    name="g_ps")
                    for d in range(D):
                        nc.tensor.matmul(
                            g_ps[:, 32 * d:32 * d + 32], wc[t][:, d, :],
                            vtp[:, 32 * d:32 * d + 32], start=True, stop=True)
                    nc.scalar.copy(
                        gtiles[t][:].rearrange("r (b d) -> r b d", b=BC),
                        g_ps[:].rearrange("r (d b) -> r b d", d=D))
                for c in range(NCH):
                    t, pos = c // 3, c % 3
                    r0 = 32 * pos
                    d_ps = psA.tile([108, BC * D], f32, tag="ps", name="d_ps")
                    for b in range(BC):
                        nc.tensor.matmul(
                            d_ps[:, D * b:D * b + D],
                            xbd2[t][r0:r0 + 24, b, :],
                            gtiles[t][r0:r0 + 24, D * b:D * b + D],
                            start=True, stop=True)
                    dst = bst[t][:, pos, :, :].rearrange("r b d -> r (b d)")
                    if it == 0:
                        nc.vector.tensor_copy(dst, d_ps[:])
                    else:
                        nc.vector.tensor_add(dst, dst, d_ps[:])

            def op_A_group(t, dst_tiles):
                # op A for the chunks of triple t; y casts on scalar so the
                # vector queue stays free for the softmax chains.
                L = TLEN[t]
                for pos in range(L):
                    c = 3 * t + pos
                    y_ps = psA.tile([24, BC * D], f32, tag="ps", name="y_ps")
                    for b in range(BC):
                        nc.tensor.matmul(
                            y_ps[:, D * b:D * b + D],
                            xbd_r[:, b, c, :],
                            ct[t][:, pos, b, :],
                            start=True, stop=True)
                    nc.scalar.copy(
                        dst_tiles[t][32 * pos:32 * pos + 24, :], y_ps[:])

            def softmax_d_group(t):
                # softmax over d on triple t's b-state -> ct[t] (bf16);
                # the exp itself is hoisted by the caller so the scalar
                # queue is not blocked behind PE-dependent casts.
                L = TLEN[t]
                zs = mpool.tile([108, L * BC], f32, tag=f"zs{t}",
                                name=f"zs{t}")
                nc.vector.tensor_reduce(zs[:], ct[t], axis=AX.X, op=ALU.add)
                zr = mpool.tile([108, L * BC], f32, tag=f"zr{t}",
                                name=f"zr{t}")
                nc.vector.reciprocal_approx_fast(zr[:], zs[:])
                eng = nc.vector if t % 2 == 0 else nc.gpsimd
                eng.tensor_mul(
                    ct[t], ct[t],
                    zr[:].rearrange("r (c b) -> r c b", c=L)
                    .broadcast_to([108, L, BC, D]))

            # ---- iteration 1 (c uniform = 1/D, folded into xs) ----
            s_ps = op_B(None, it=0)
            v = squash(s_ps)
            op_CD(v, it=0)

            # ---- iteration 2: softmax over d, pipelined per triple ----
            for t in range(NT):
                nc.scalar.activation(ctile[t][:], bstate[t][:], AF.Exp)
            for t in range(NT):
                softmax_d_group(t)
                op_A_group(t, ytiles)
            s_ps = op_B(ytiles, it=1)
            v = squash(s_ps)
            op_CD(v, it=1)

            # ---- final: softmax over p fused into op A, per triple ----
            for t in range(NT):
                nc.scalar.activation(ctile[t][:], bstate[t][:], AF.Exp)
            for t in range(NT):
                L = TLEN[t]
                for pos in range(L):
                    z_ps = psA.tile([24, BC * D], f32, tag="ps", name="z_ps")
                    nc.tensor.matmul(
                        z_ps[:], sb["ones_bd"][:],
                        ct[t][:, pos, :, :].rearrange("r b d -> r (b d)"),
                        start=True, stop=True)
                    nc.vector.tensor_copy(
                        ztiles[t][32 * pos:32 * pos + 24, :], z_ps[:])
                op_A_group(t, ytiles)
                zrt = spool.tile([TROWS[t], BC * D], f32, tag=f"zr_t{t}",
                                 name=f"zr_t{t}")
                nc.vector.reciprocal_approx_fast(zrt[:], ztiles[t][:])
                nc.gpsimd.tensor_mul(ytiles[t][:], ytiles[t][:], zrt[:])
            s_ps = op_B(ytiles, it=2)
            v = squash(s_ps)
            nc.sync.dma_start(out_d[:], v[:])
    return nc


_CACHE = {}


def kernel(x, W):
    import sys
    if "/opt/trn_rl_repo" not in sys.path:
        sys.path.insert(0, "/opt/trn_rl_repo")
    from concourse import bass_utils

    x = np.asarray(x, np.float32)
    Wd = np.asarray(W, np.float32)[0, :, :, 0]  # [D,M,O,I]
    if "nc" not in _CACHE:
        from concourse import bacc
        nc = _build(bacc.Bacc(None, target_bir_lowering=False))
        nc.compile()
        _CACHE["nc"] = nc
    nc = _CACHE["nc"]
    in_maps = [_host_prep(x[k * BC:(k + 1) * BC], Wd) for k in range(NCORES)]
    res = bass_utils.run_bass_kernel_spmd(nc, in_maps, list(range(NCORES)))
    outs = [res.results[k]["out_v"].reshape(BC, D, O) for k in range(NCORES)]
    return np.concatenate(outs, axis=0)


# revision 11
# speedup vs baseline: 1.0451x; 1.0336x over previous
"""CapsNet dynamic-routing kernel for Trainium2 (8 NeuronCores, batch-parallel).

Restructured routing that never materializes u_hat (B=256,D=10,M=32,P=36,I=8,O=16):
  y[b,d,m,i] = sum_p c[b,d,m,p] x[b,m,p,i]        (op A, PE per (b,chunk))
  s[b,d,o]   = sum_{m,i} y[b,d,m,i] W[d,m,o,i]    (op B)
  g[b,d,m,i] = sum_o W[d,m,o,i] v[b,d,o]          (op C)
  b[b,d,m,p]+= sum_i x[b,p,i] g[b,d,m,i]          (op D, PE per (b,chunk))

m is split into 11 chunks of 3, grouped in 4 triples t (chunks 3t..3t+L-1).
b-state/c live on SBUF rows (mc,p)=108, one tile per triple with cols
(pos,b,d). y/g/z rows are (mc,i)=24 per chunk at row offsets 0/32/64 with
zero padding rows. All x-derived matmul operands are prepared host-side and
shipped in a handful of packed DMAs (small operands first so op B of the
uniform-c iteration starts immediately).

PE operands are bf16 (fp32 matmuls cost 2x LDWEIGHTS+MATMUL passes); PSUM
accumulation, b-state and squash stay fp32. The scalar engine only uses
Copy/Square/Ln/Exp; sqrt comes from exp(-.5 ln). A dummy ln issued right
after each softmax's exps pulls the ln-side activation-table load off the
squash critical path. Softmaxes are processed per-triple so the PE pipeline
overlaps the vector work; PSUM->SBUF casts run on scalar, softmax divides
use the fast DVE reciprocal approximation, and the normalization multiplies
alternate vector/gpsimd.
"""

import numpy as np
import ml_dtypes

B, D, M, P, I, O = 256, 10, 32, 36, 8, 16
NCORES = 8
BC = B // NCORES
NCH = 11
NT = 4                      # triples of chunks: [0-2],[3-5],[6-8],[9-10]
TLEN = [3, 3, 3, 2]
TROWS = [96, 96, 96, 64]    # 32 rows per chunk (24 real + 8 zero pad)
EPS = 1e-7

BF16 = ml_dtypes.bfloat16

# packed free-dim offsets (in elements) for the merged DMA tensors
SW_XS = [t * 32 for t in range(NT)]                     # [96, 128]
SW_WS = [128 + t * D * O for t in range(NT)]            # [96, 128+640]
SW_COLS = 128 + NT * D * O
WC_OFF = [0, 960, 1920, 2880]                           # [16, 3520]
WC_COLS = 3520
X2_OFF = [t * BC * 3 * P for t in range(NT)]            # [96, 4*3456]
X2_COLS = NT * BC * 3 * P


def _host_prep(xc, Wd):
    """Per-core host-side tensor prep. xc: [BC,M,P,I], Wd: [D,M,O,I]."""
    f32 = np.float32
    xbd = np.zeros((3, P, BC, NCH, 3, I), f32)
    xsum = xc.sum(axis=2) * (1.0 / D)
    small = np.zeros((96, SW_COLS), f32)
    wcp = np.zeros((O, WC_COLS), f32)
    x2p = np.zeros((96, X2_COLS), f32)
    for t in range(NT):
        L = TLEN[t]
        rows = TROWS[t]
        t2 = np.zeros((rows, BC, 3, P), f32)
        tw = np.zeros((rows, D, O), f32)
        tcc = np.zeros((O, D, L, 4, I), f32)
        txs = np.zeros((rows, BC), f32)
        for pos in range(L):
            c = 3 * t + pos
            r0 = 32 * pos
            for mc in range(3):
                m = 3 * c + mc
                if m >= M:
                    continue
                xmi = xc[:, m, :, :]                      # [b, p, i]
                xbd[mc, :, :, c, mc, :] = xmi.transpose(1, 0, 2)
                rr = r0 + 8 * mc
                t2[rr:rr + 8, :, mc, :] = xmi.transpose(2, 0, 1)
                tw[rr:rr + 8, :, :] = Wd[:, m, :, :].transpose(2, 0, 1)
                tcc[:, :, pos, mc, :] = Wd[:, m, :, :].transpose(1, 0, 2)
                txs[rr:rr + 8, :] = xsum[:, m, :].T
        small[:rows, SW_XS[t]:SW_XS[t] + BC] = txs
        small[:rows, SW_WS[t]:SW_WS[t] + D * O] = tw.reshape(rows, D * O)
        wcp[:, WC_OFF[t]:WC_OFF[t] + D * L * 32] = tcc.reshape(O, D * L * 32)
        x2p[:rows, X2_OFF[t]:X2_OFF[t] + BC * 3 * P] = t2.reshape(
            rows, BC * 3 * P)
    out = {
        "smallp": small.astype(BF16),
        "wcp": wcp.astype(BF16),
        "x2p": np.ascontiguousarray(x2p.astype(BF16)),
        "xbd": np.ascontiguousarray(
            xbd.reshape(108, BC * NCH * 24).astype(BF16)),
        "ident": np.eye(32, dtype=f32),
    }
    ones_bd = np.zeros((3, P, 3, I), f32)
    for mc in range(3):
        ones_bd[mc, :, mc, :] = 1.0
    out["ones_bd"] = ones_bd.reshape(108, 24).astype(BF16)
    return out


def _build(nc):
    import concourse.mybir as mybir
    import concourse.tile as tile

    f32 = mybir.dt.float32
    i32 = mybir.dt.int32
    bf16 = mybir.dt.bfloat16
    AF = mybir.ActivationFunctionType
    AX = mybir.AxisListType
    ALU = mybir.AluOpType

    ins = {}
    # declaration order == DMA issue order: small it0-critical operands
    # first, then the two big x-derived tensors.
    specs = [
        ("smallp", [96, SW_COLS], bf16),
        ("wcp", [O, WC_COLS], bf16),
        ("ones_bd", [108, 24], bf16),
        ("ident", [32, 32], f32),
        ("x2p", [96, X2_COLS], bf16),
        ("xbd", [108, BC * NCH * 24], bf16),
    ]
    for name, shape, dt in specs:
        ins[name] = nc.declare_dram_parameter(name, shape, dt, isOutput=False)
    out_d = nc.declare_dram_parameter("out_v", [BC, D * O], f32, isOutput=True)

    with tile.TileContext(nc) as tc:
        with (
            tc.tile_pool(name="const", bufs=1) as cpool,
            tc.tile_pool(name="state", bufs=1) as spool,
            tc.tile_pool(name="small", bufs=2) as mpool,
            tc.tile_pool(name="psA", bufs=4, space="PSUM") as psA,
            tc.tile_pool(name="psS", bufs=1, space="PSUM") as psS,
        ):
            sb = {}
            for name, t in ins.items():
                st = cpool.tile(list(t.shape), t.dtype, name=name, tag=name)
                nc.sync.dma_start(st[:], t[:])
                sb[name] = st

            xbd_r = sb["xbd"][:].rearrange("r (b c k) -> r b c k", b=BC, c=NCH)
            xbd2 = [sb["x2p"][0:TROWS[t], X2_OFF[t]:X2_OFF[t] + BC * 3 * P]
                    .rearrange("r (b q) -> r b q", b=BC) for t in range(NT)]
            ws = [sb["smallp"][0:TROWS[t], SW_WS[t]:SW_WS[t] + D * O]
                  .rearrange("r (d o) -> r d o", d=D) for t in range(NT)]
            wc = [sb["wcp"][:, WC_OFF[t]:WC_OFF[t] + D * TLEN[t] * 32]
                  .rearrange("o (d r) -> o d r", d=D) for t in range(NT)]
            xs = [sb["smallp"][0:TROWS[t], SW_XS[t]:SW_XS[t] + BC]
                  for t in range(NT)]

            # per-triple routing state: [108, L*BC*D]
            bstate = [spool.tile([108, TLEN[t] * BC * D], f32,
                                 name=f"bst{t}", tag=f"bst{t}")
                      for t in range(NT)]
            bst = [bstate[t][:].rearrange("r (c b d) -> r c b d",
                                          c=TLEN[t], b=BC)
                   for t in range(NT)]
            ctile = [spool.tile([108, TLEN[t] * BC * D], bf16,
                                name=f"ct{t}", tag=f"ct{t}")
                     for t in range(NT)]
            ct = [ctile[t][:].rearrange("r (c b d) -> r c b d",
                                        c=TLEN[t], b=BC)
                  for t in range(NT)]
            ytiles = [spool.tile([TROWS[t], BC * D], bf16, tag=f"y{t}",
                                 name=f"y{t}") for t in range(NT)]
            gtiles = [spool.tile([TROWS[t], BC * D], bf16, tag=f"g{t}",
                                 name=f"g{t}") for t in range(NT)]
            ztiles = [spool.tile([TROWS[t], BC * D], f32, tag=f"z{t}",
                                 name=f"z{t}") for t in range(NT)]
            for t in range(NT):
                nc.gpsimd.memset(ytiles[t][:], 0.0)   # pad rows must stay 0
                nc.gpsimd.memset(ztiles[t][:], 1.0)   # pad rows must stay 1

            def op_B_part(src_y, it, t):
                # partial s for triple t into its own PSUM tile; emitted
                # right after op_A of the same triple so the PE never drains
                sp = psS.tile([BC, D * O], f32, tag=f"s{t}", name=f"s{t}")
                for d in range(D):
                    if it == 0:
                        lhsT = xs[t]
                    else:
                        lhsT = src_y[t][:].rearrange(
                            "r (d b) -> r d b", d=D)[:, d, :]
                    nc.tensor.matmul(
                        sp[:, d * O:(d + 1) * O], lhsT, ws[t][:, d, :],
                        start=True, stop=True)
                return sp

            def op_B_sum(parts):
                # DVE may read only one PSUM operand per op: accumulate
                # into SBUF, with the scalar engine handling one copy.
                s01 = mpool.tile([BC, D * O], f32, tag="s01", name="s01")
                nc.scalar.copy(s01[:], parts[0][:])
                s23 = mpool.tile([BC, D * O], f32, tag="s23", name="s23")
                nc.vector.tensor_copy(s23[:], parts[2][:])
                nc.vector.tensor_add(s01[:], s01[:], parts[1][:])
                nc.vector.tensor_add(s23[:], s23[:], parts[3][:])
                s_sb = mpool.tile([BC, D * O], f32, tag="ssb", name="s_sb")
                nc.vector.tensor_add(s_sb[:], s01[:], s23[:])
                return s_sb

            def squash(s_sb):
                # v = s * ssum / ((1+ssum) sqrt(ssum+eps)); rsqrt on DVE
                sq = mpool.tile([BC, D * O], f32, tag="sq", name="sq")
                nc.scalar.activation(sq[:], s_sb[:], AF.Square)
                ssum = mpool.tile([BC, D], f32, tag="ssum", name="ssum")
                nc.vector.tensor_reduce(
                    ssum[:], sq[:].rearrange("b (d o) -> b d o", d=D),
                    axis=AX.X, op=ALU.add)
                se = mpool.tile([BC, D], f32, tag="se", name="se")
                nc.vector.tensor_scalar_add(se[:], ssum[:], EPS)
                # rs = rsqrt(se) via shift/magic seed + 2 Newton steps --
                # all on DVE, so the scalar engine never leaves the exp
                # activation-table set (zero mid-kernel table reloads).
                rs = mpool.tile([BC, D], f32, tag="rs", name="rs")
                nc.vector.tensor_scalar(
                    rs[:].bitcast(i32), se[:].bitcast(i32), 1, None,
                    op0=ALU.logical_shift_right)
                nc.vector.tensor_scalar(
                    rs[:].bitcast(i32), rs[:].bitcast(i32), -1, 0x5F3759DF,
                    op0=ALU.mult, op1=ALU.add)
                nt = mpool.tile([BC, D], f32, tag="nt", name="nt")
                for _ in range(1):
                    nc.vector.tensor_mul(nt[:], rs[:], rs[:])
                    nc.vector.tensor_mul(nt[:], nt[:], se[:])
                    nc.vector.tensor_scalar(nt[:], nt[:], -0.5, 1.5,
                                            op0=ALU.mult, op1=ALU.add)
                    nc.vector.tensor_mul(rs[:], rs[:], nt[:])
                den = mpool.tile([BC, D], f32, tag="den", name="den")
                nc.vector.tensor_scalar_add(den[:], ssum[:], 1.0)
                rden = mpool.tile([BC, D], f32, tag="rden", name="rden")
                nc.vector.reciprocal_approx_fast(rden[:], den[:])
                sc = mpool.tile([BC, D], f32, tag="sc", name="sc")
                nc.vector.tensor_mul(sc[:], ssum[:], rden[:])
                nc.vector.tensor_mul(sc[:], sc[:], rs[:])
                v = mpool.tile([BC, D * O], f32, tag="v", name="v")
                nc.vector.tensor_mul(
                    v[:].rearrange("b (d o) -> b d o", d=D),
                    s_sb[:].rearrange("b (d o) -> b d o", d=D),
                    sc[:].broadcast_to([BC, D, O]))
                return v

            def op_CD(v, it):
                # stage VT as [o=16, (d,b)] so matmul reads start at partition 0
                vtp = mpool.tile([O, D * 32], bf16, tag="vtp", name="vtp")
                for d in range(D):
                    vt_ps = psA.tile([O, 32], f32, tag="ps", name="vt_ps")
                    nc.tensor.transpose(
                        vt_ps[:], v[:, 16 * d:16 * d + 16], sb["ident"][:])
                    nc.vector.tensor_copy(vtp[:, 32 * d:32 * d + 32], vt_ps[:])
                for t in range(NT):
                    L = TLEN[t]
                    g_ps = psA.tile([32 * L, D * 32], f32, tag="ps",
                                    name="g_ps")
                    for d in range(D):
                        nc.tensor.matmul(
                            g_ps[:, 32 * d:32 * d + 32], wc[t][:, d, :],
                            vtp[:, 32 * d:32 * d + 32], start=True, stop=True)
                    nc.scalar.copy(
                        gtiles[t][:].rearrange("r (b d) -> r b d", b=BC),
                        g_ps[:].rearrange("r (d b) -> r b d", d=D))
                for c in range(NCH):
                    t, pos = c // 3, c % 3
                    r0 = 32 * pos
                    d_ps = psA.tile([108, BC * D], f32, tag="ps", name="d_ps")
                    for b in range(BC):
                        nc.tensor.matmul(
                            d_ps[:, D * b:D * b + D],
                            xbd2[t][r0:r0 + 24, b, :],
                            gtiles[t][r0:r0 + 24, D * b:D * b + D],
                            start=True, stop=True)
                    dst = bst[t][:, pos, :, :].rearrange("r b d -> r (b d)")
                    if it == 0:
                        nc.vector.tensor_copy(dst, d_ps[:])
                    else:
                        nc.vector.tensor_add(dst, dst, d_ps[:])

            def op_A_group(t, dst_tiles):
                # op A for the chunks of triple t; y casts on scalar so the
                # vector queue stays free for the softmax chains.
                L = TLEN[t]
                for pos in range(L):
                    c = 3 * t + pos
                    y_ps = psA.tile([24, BC * D], f32, tag="ps", name="y_ps")
                    for b in range(BC):
                        nc.tensor.matmul(
                            y_ps[:, D * b:D * b + D],
                            xbd_r[:, b, c, :],
                            ct[t][:, pos, b, :],
                            start=True, stop=True)
                    nc.scalar.copy(
                        dst_tiles[t][32 * pos:32 * pos + 24, :]
                        .rearrange("r (d b) -> r d b", d=D),
                        y_ps[:].rearrange("r (b d) -> r d b", d=D))

            def softmax_d_group(t):
                # softmax over d on triple t's b-state -> ct[t] (bf16);
                # the exp itself is hoisted by the caller so the scalar
                # queue is not blocked behind PE-dependent casts.
                L = TLEN[t]
                zs = mpool.tile([108, L * BC], f32, tag=f"zs{t}",
                                name=f"zs{t}")
                nc.vector.tensor_reduce(zs[:], ct[t], axis=AX.X, op=ALU.add)
                zr = mpool.tile([108, L * BC], f32, tag=f"zr{t}",
                                name=f"zr{t}")
                nc.vector.reciprocal_approx_fast(zr[:], zs[:])
                eng = nc.vector if t % 2 == 0 else nc.gpsimd
                eng.tensor_mul(
                    ct[t], ct[t],
                    zr[:].rearrange("r (c b) -> r c b", c=L)
                    .broadcast_to([108, L, BC, D]))

            # ---- iteration 1 (c uniform = 1/D, folded into xs) ----
            parts = [op_B_part(None, 0, t) for t in range(NT)]
            v = squash(op_B_sum(parts))
            op_CD(v, it=0)

            # ---- iteration 2: softmax over d, pipelined per triple ----
            for t in range(NT):
                nc.scalar.activation(ctile[t][:], bstate[t][:], AF.Exp)
            parts = []
            for t in range(NT):
                softmax_d_group(t)
                op_A_group(t, ytiles)
                parts.append(op_B_part(ytiles, 1, t))
            v = squash(op_B_sum(parts))
            op_CD(v, it=1)

            # ---- final: softmax over p fused into op A, per triple ----
            for t in range(NT):
                nc.scalar.activation(ctile[t][:], bstate[t][:], AF.Exp)
            parts = []
            for t in range(NT):
                L = TLEN[t]
                for pos in range(L):
                    z_ps = psA.tile([24, BC * D], f32, tag="ps", name="z_ps")
                    nc.tensor.matmul(
                        z_ps[:], sb["ones_bd"][:],
                        ct[t][:, pos, :, :].rearrange("r b d -> r (b d)"),
                        start=True, stop=True)
                    nc.vector.tensor_copy(
                        ztiles[t][32 * pos:32 * pos + 24, :]
                        .rearrange("r (d b) -> r d b", d=D),
                        z_ps[:].rearrange("r (b d) -> r d b", d=D))
                op_A_group(t, ytiles)
                zrt = spool.tile([TROWS[t], BC * D], f32, tag=f"zr_t{t}",
                                 name=f"zr_t{t}")
                nc.vector.reciprocal_approx_fast(zrt[:], ztiles[t][:])
                nc.gpsimd.tensor_mul(ytiles[t][:], ytiles[t][:], zrt[:])
                parts.append(op_B_part(ytiles, 2, t))
            v = squash(op_B_sum(parts))
            nc.sync.dma_start(out_d[:], v[:])
    return nc


_CACHE = {}


def kernel(x, W):
    import sys
    if "/opt/trn_rl_repo" not in sys.path:
        sys.path.insert(0, "/opt/trn_rl_repo")
    from concourse import bass_utils

    x = np.asarray(x, np.float32)
    Wd = np.asarray(W, np.float32)[0, :, :, 0]  # [D,M,O,I]
    if "nc" not in _CACHE:
        from concourse import bacc
        nc = _build(bacc.Bacc(None, target_bir_lowering=False))
        nc.compile()
        _CACHE["nc"] = nc
    nc = _CACHE["nc"]
    in_maps = [_host_prep(x[k * BC:(k + 1) * BC], Wd) for k in range(NCORES)]
    res = bass_utils.run_bass_kernel_spmd(nc, in_maps, list(range(NCORES)))
    outs = [res.results[k]["out_v"].reshape(BC, D, O) for k in range(NCORES)]
    return np.concatenate(outs, axis=0)
